# revision 10
# baseline (speedup 1.0000x reference)
"""Trainium2 Bass kernel for nn_CrossAttention (B=4, N=4096, T=256, DIM=1024,
16 heads x 64 dim, cosine-sim attention with null-kv token, LN in/ctx/out).

Sharding: data-parallel over query rows. Core c handles batch b=c//2, query
rows (c%2)*2048 : (c%2)*2048+2048. The kv projections (tiny: T=256) are
computed redundantly per core; no collectives are needed. Each core returns
its [2048, 1024] output slice; the host reassembles the full [4,4096,1024].

All matmuls run with fp16 inputs (fp32 PSUM accumulation) at 1 PE cycle/row;
fp32/f32r matmuls crash the exec unit on this runtime and bf16 would cost 8x
mantissa. The attention inputs are l2-normalized so fp16 conditioning is
ideal. Statistics (LN mean/var, l2 norms, softmax sums, reciprocals) are all
computed in fp32 from fp32 PSUM values; rounding to fp16 happens only on
matmul operands.

Dataflow per core:
  - LN(x) in fp32 -> xn (fp16) -> PE-transpose -> xnT
  - q = xnT @ Wq (fp32 psum), l2-normalize per head along free dim (fp32
    stats), write q_final fp16 -> PE-transpose -> qT [dim, 512 q]
  - scoresT_h = kT_h^T @ qT_h in [kv, q] layout (kv on partitions) so softmax
    needs no partition reductions: exp via ACT with fused scale=8 and bias
    ln(1/256) (keeps fp16 exp values <= ~12; the bias cancels exactly in the
    softmax division). The kv-sum comes free as a ones-column appended to v
    (row 64 of the outT psum); 1/sum is broadcast across partitions with a
    K=1 ones matmul + DVE reciprocal.
  - outT_h accumulated unnormalized, scaled by 1/sum at the PSUM->SBUF copy
    into head-pair layout [128, 8, 512] which feeds to_out directly as lhsT
    (odd heads staged through SBUF and partition-shifted by DMA).
  - final = outT^T @ Wo accumulated over 8 K-chunks (fp32 psum), then LN(out)
    in fp32.
"""

import numpy as np
from contextlib import ExitStack

import concourse.bass as bass
import concourse.tile as tile
from concourse import bacc, mybir
from concourse.bass_utils import run_bass_kernel_spmd
from concourse.masks import make_identity

F32 = mybir.dt.float32
F16 = mybir.dt.float16
AF = mybir.ActivationFunctionType
AX = mybir.AxisListType

DIM = 1024
HEADS = 16
HD = 64
T = 256
TK = T + 1  # with null token
SCALE = 8.0
EXPB = -5.545177444479562  # ln(1/256); cancels in softmax, keeps fp16 in range
LN_EPS = 1e-5
NORM_EPS = 1e-12
N_CORES = 8
ROWS = 2048          # query rows per core
QMACRO = 512         # rows per macro tile
NSUB = QMACRO // 128  # 4 subtiles per macro
NMACRO = ROWS // QMACRO  # 4


def _emit_layernorm(nc, pool_small, out_ap, in_ap, g_tile, b_tile, eps_tile):
    """out = (in - mean)/sqrt(var+eps) * g + b, row-wise over 1024 free dim."""
    stats = pool_small.tile([128, 2, 6], F32, tag="lnstats")
    nc.vector.bn_stats(out=stats[:, 0, :], in_=in_ap[:, 0:512])
    nc.vector.bn_stats(out=stats[:, 1, :], in_=in_ap[:, 512:1024])
    mv = pool_small.tile([128, 2], F32, tag="lnmv")
    nc.vector.bn_aggr(out=mv[:], in_=stats[:])
    std = pool_small.tile([128, 1], F32, tag="lnstd")
    nc.scalar.activation(out=std[:], in_=mv[:, 1:2], func=AF.Sqrt,
                         bias=eps_tile[:], scale=1.0)
    rstd = pool_small.tile([128, 1], F32, tag="lnrstd")
    nc.vector.reciprocal(rstd[:], std[:])
    negmr = pool_small.tile([128, 1], F32, tag="lnnegmr")
    nc.vector.tensor_mul(negmr[:], mv[:, 0:1], rstd[:])
    nc.vector.tensor_scalar_mul(negmr[:], negmr[:], -1.0)
    nc.scalar.activation(out=out_ap, in_=in_ap, func=AF.Identity,
                         bias=negmr[:], scale=rstd[:])
    nc.vector.tensor_mul(out_ap, out_ap, g_tile[:])
    nc.vector.tensor_add(out_ap, out_ap, b_tile[:])


def _emit_l2norm_heads(nc, pool_small, sq_pool, out_ap3, in_ap, scale_tile):
    """in_ap: [128, 1024] fp32 (psum). out_ap3: [128, 16, 64] sbuf AP (fp16).
    out = in / max(||in_head||, eps) * scale (per head of 64)."""
    sq = sq_pool.tile([128, 1024], F32, tag="sq")
    nc.scalar.activation(out=sq[:], in_=in_ap, func=AF.Square, bias=0.0, scale=1.0)
    ssq = pool_small.tile([128, 16], F32, tag="ssq")
    nc.vector.reduce_sum(out=ssq[:], in_=sq[:].rearrange("p (h d) -> p h d", d=HD),
                         axis=AX.X)
    norm = pool_small.tile([128, 16], F32, tag="l2norm")
    nc.scalar.activation(out=norm[:], in_=ssq[:], func=AF.Sqrt, bias=0.0, scale=1.0)
    nc.vector.tensor_scalar_max(norm[:], norm[:], NORM_EPS)
    rn = pool_small.tile([128, 16], F32, tag="l2rn")
    nc.vector.reciprocal(rn[:], norm[:])
    in3 = in_ap.rearrange("p (h d) -> p h d", d=HD)
    nc.vector.tensor_mul(out_ap3, in3, rn[:].unsqueeze(-1).broadcast_to([128, 16, HD]))
    nc.vector.tensor_mul(out_ap3, out_ap3,
                         scale_tile[:].unsqueeze(1).broadcast_to([128, 16, HD]))


def _load_bcast(nc, dst_tile, dram_ap, parts=128):
    """DMA-load a [F] dram vector replicated across `parts` partitions."""
    ap = bass.AP(tensor=dram_ap.tensor, offset=dram_ap.offset,
                 ap=[[0, parts]] + dram_ap.ap)
    nc.sync.dma_start(out=dst_tile[:parts, :], in_=ap)


def build_nc():
    nc = bacc.Bacc("TRN2", debug=False)

    XS = nc.dram_tensor("xs", [ROWS, DIM], F32, kind="ExternalInput")
    CTX = nc.dram_tensor("ctx", [T, DIM], F32, kind="ExternalInput")
    WQ = nc.dram_tensor("Wq", [DIM, DIM], F16, kind="ExternalInput")
    WKV = nc.dram_tensor("Wkv", [DIM, 2 * DIM], F16, kind="ExternalInput")
    WO = nc.dram_tensor("Wo", [DIM, DIM], F16, kind="ExternalInput")
    NKV = nc.dram_tensor("null_kv", [2, HD], F32, kind="ExternalInput")
    QS = nc.dram_tensor("q_scale", [HD], F32, kind="ExternalInput")
    KS = nc.dram_tensor("k_scale", [HD], F32, kind="ExternalInput")
    LIG = nc.dram_tensor("ln_in_g", [DIM], F32, kind="ExternalInput")
    LIB = nc.dram_tensor("ln_in_b", [DIM], F32, kind="ExternalInput")
    LCG = nc.dram_tensor("ln_ctx_g", [DIM], F32, kind="ExternalInput")
    LCB = nc.dram_tensor("ln_ctx_b", [DIM], F32, kind="ExternalInput")
    LOG = nc.dram_tensor("ln_out_g", [DIM], F32, kind="ExternalInput")
    LOB = nc.dram_tensor("ln_out_b", [DIM], F32, kind="ExternalInput")
    OUT = nc.dram_tensor("out", [ROWS, DIM], F32, kind="ExternalOutput")

    with tile.TileContext(nc) as tc, ExitStack() as ctx:
        # ---------------- persistent pools ----------------
        consts = ctx.enter_context(tc.tile_pool(name="consts", bufs=1))
        weights = ctx.enter_context(tc.tile_pool(name="weights", bufs=1))
        kvpool = ctx.enter_context(tc.tile_pool(name="kvpool", bufs=1))
        small = ctx.enter_context(tc.tile_pool(name="small", bufs=2))
        sq_pool = ctx.enter_context(tc.tile_pool(name="sqp", bufs=1))

        ps_tr = ctx.enter_context(tc.tile_pool(name="ps_tr", bufs=1, space="PSUM"))
        ps_big = ctx.enter_context(tc.tile_pool(name="ps_big", bufs=1, space="PSUM"))
        ps_sc = ctx.enter_context(tc.tile_pool(name="ps_sc", bufs=1, space="PSUM"))
        ps_sm = ctx.enter_context(tc.tile_pool(name="ps_sm", bufs=1, space="PSUM"))
        ps_o = ctx.enter_context(tc.tile_pool(name="ps_o", bufs=2, space="PSUM"))

        ident = consts.tile([128, 128], F16)
        make_identity(nc, ident)
        eps_tile = consts.tile([128, 1], F32)
        nc.vector.memset(eps_tile[:], LN_EPS)
        onesf = consts.tile([128, 1], F32)
        nc.vector.memset(onesf[:], 1.0)
        expb = consts.tile([128, 1], F32)
        nc.vector.memset(expb[:], EXPB)
        ones_t = consts.tile([128, HD], F16)
        nc.vector.tensor_copy(ones_t[:], onesf[:, 0:1].broadcast_to([128, HD]))

        lig = consts.tile([128, DIM], F32)
        lib_ = consts.tile([128, DIM], F32)
        log_ = consts.tile([128, DIM], F32)
        lob = consts.tile([128, DIM], F32)
        _load_bcast(nc, lig, LIG[:])
        _load_bcast(nc, lib_, LIB[:])
        _load_bcast(nc, log_, LOG[:])
        _load_bcast(nc, lob, LOB[:])
        qsc = consts.tile([128, HD], F32)
        _load_bcast(nc, qsc, QS[:])
        ksc = consts.tile([128, HD], F32)
        _load_bcast(nc, ksc, KS[:])

        wq_sb = weights.tile([128, 8, DIM], F16)
        for kc in range(8):
            nc.sync.dma_start(out=wq_sb[:, kc, :], in_=WQ[kc * 128:(kc + 1) * 128, :])
        wo_sb = weights.tile([128, 8, DIM], F16)
        for kc in range(8):
            nc.sync.dma_start(out=wo_sb[:, kc, :], in_=WO[kc * 128:(kc + 1) * 128, :])

        # kT: [dim-in-pair 128, pair 8, token 257]; v': [kv 128, chunk 2, head 16, 65]
        kT = kvpool.tile([128, 8, TK], F16)
        v_sb = kvpool.tile([128, 2, HEADS, HD + 1], F16)
        nc.vector.tensor_copy(
            v_sb[:, :, :, HD:HD + 1],
            onesf[:, 0:1].unsqueeze(1).unsqueeze(1).broadcast_to([128, 2, HEADS, 1]))
        vnull = kvpool.tile([1, HD + 1], F16)

        # ---------------- phase K: context -> kT, v' ----------------
        with ExitStack() as kctx:
            pk = kctx.enter_context(tc.tile_pool(name="pk", bufs=2))
            pk1 = kctx.enter_context(tc.tile_pool(name="pk1", bufs=1))

            lcg = pk1.tile([128, DIM], F32, tag="lcparam")
            lcb = pk1.tile([128, DIM], F32, tag="lcparam2")
            _load_bcast(nc, lcg, LCG[:])
            _load_bcast(nc, lcb, LCB[:])

            cnT = pk1.tile([128, 8, T], F16)
            for i in range(2):
                ctx_t = pk.tile([128, DIM], F32, tag="ctx")
                nc.sync.dma_start(out=ctx_t[:], in_=CTX[i * 128:(i + 1) * 128, :])
                cn = pk.tile([128, DIM], F16, tag="cn")
                _emit_layernorm(nc, small, cn[:], ctx_t[:], lcg, lcb, eps_tile)
                for c0 in (0, 4):
                    ptr = ps_tr.tile([128, 4, 128], F16, tag="tr")
                    for t in range(4):
                        nc.tensor.transpose(ptr[:, t, :],
                                            cn[:, (c0 + t) * 128:(c0 + t + 1) * 128],
                                            ident[:])
                    nc.vector.tensor_copy(cnT[:, c0:c0 + 4, i * 128:(i + 1) * 128],
                                          ptr[:])

            # k and v projections (separate passes; ps_big has 1 buf)
            for i in range(2):
                for which in (0, 1):  # 0 = k, 1 = v
                    pb = ps_big.tile([128, DIM], F32, tag="big")
                    for kc in range(8):
                        wkv_c = pk.tile([128, DIM], F16, tag="wkv")
                        nc.sync.dma_start(
                            out=wkv_c[:],
                            in_=WKV[kc * 128:(kc + 1) * 128,
                                    which * DIM:(which + 1) * DIM])
                        for half in range(2):
                            nc.tensor.matmul(
                                pb[:, half * 512:(half + 1) * 512],
                                lhsT=cnT[:, kc, i * 128:(i + 1) * 128],
                                rhs=wkv_c[:, half * 512:(half + 1) * 512],
                                start=(kc == 0), stop=(kc == 7))
                    if which == 0:
                        kfin = pk.tile([128, DIM], F16, tag="kfin")
                        _emit_l2norm_heads(nc, small, sq_pool,
                                           kfin[:].rearrange("p (h d) -> p h d", d=HD),
                                           pb[:], ksc)
                        for c0 in (0, 4):
                            ptr = ps_tr.tile([128, 4, 128], F16, tag="tr")
                            for t in range(4):
                                nc.tensor.transpose(
                                    ptr[:, t, :],
                                    kfin[:, (c0 + t) * 128:(c0 + t + 1) * 128],
                                    ident[:])
                            nc.vector.tensor_copy(
                                kT[:, c0:c0 + 4, i * 128:(i + 1) * 128], ptr[:])
                    else:
                        nc.vector.tensor_copy(
                            v_sb[:, i, :, 0:HD],
                            pb[:].rearrange("p (h d) -> p h d", d=HD))

            # null kv token
            nkv = pk1.tile([1, 2, HD], F32)
            nc.sync.dma_start(out=nkv[0:1, :, :], in_=NKV[:, :])
            ksc0 = pk1.tile([1, HD], F32)
            nc.sync.dma_start(out=ksc0[0:1, :], in_=KS[:])
            sqn = pk1.tile([1, HD], F32)
            nc.vector.tensor_mul(sqn[0:1, :], nkv[0:1, 0, :], nkv[0:1, 0, :])
            ssqn = pk1.tile([1, 1], F32)
            nc.vector.reduce_sum(out=ssqn[0:1, :], in_=sqn[0:1, :], axis=AX.X)
            nc.scalar.activation(out=ssqn[0:1, :], in_=ssqn[0:1, :], func=AF.Sqrt,
                                 bias=0.0, scale=1.0)
            nc.vector.tensor_scalar_max(ssqn[0:1, :], ssqn[0:1, :], NORM_EPS)
            rnn = pk1.tile([1, 1], F32)
            nc.vector.reciprocal(rnn[0:1, :], ssqn[0:1, :])
            knf = pk1.tile([1, HD], F32)
            nc.vector.tensor_mul(knf[0:1, :], nkv[0:1, 0, :],
                                 rnn[0:1, 0:1].broadcast_to([1, HD]))
            nc.vector.tensor_mul(knf[0:1, :], knf[0:1, :], ksc0[0:1, :])
            kn16 = pk1.tile([1, HEADS, HD], F16)
            nc.vector.tensor_copy(kn16[0:1, :, :],
                                  knf[0:1, :].unsqueeze(1).broadcast_to([1, HEADS, HD]))
            kn16f = kn16[0:1, :, :].rearrange("p h d -> p (h d)")
            ptr = ps_tr.tile([128, 8, 2], F16, tag="tr")
            for c in range(8):
                nc.tensor.transpose(ptr[:, c, 0:1],
                                    kn16f[0:1, c * 128:(c + 1) * 128],
                                    ident[0:1, 0:1])
            nc.vector.tensor_copy(kT[:, :, T:T + 1], ptr[:, :, 0:1])
            # v'null row: [v_null | 1]
            nc.vector.tensor_copy(vnull[0:1, 0:HD], nkv[0:1, 1, :])
            nc.vector.tensor_copy(vnull[0:1, HD:HD + 1], onesf[0:1, 0:1])

        # ---------------- main loop pools ----------------
        xin = ctx.enter_context(tc.tile_pool(name="xin", bufs=2))
        xnp = ctx.enter_context(tc.tile_pool(name="xnp", bufs=2))
        xnTp = ctx.enter_context(tc.tile_pool(name="xnTp", bufs=2))
        qfp = ctx.enter_context(tc.tile_pool(name="qfp", bufs=2))
        qTp = ctx.enter_context(tc.tile_pool(name="qTp", bufs=1))
        etp = ctx.enter_context(tc.tile_pool(name="etp", bufs=2))
        etn = ctx.enter_context(tc.tile_pool(name="etn", bufs=2))
        sumsp = ctx.enter_context(tc.tile_pool(name="sumsp", bufs=2))
        rbp = ctx.enter_context(tc.tile_pool(name="rbp", bufs=2))
        stgp = ctx.enter_context(tc.tile_pool(name="stgp", bufs=2))
        outTp = ctx.enter_context(tc.tile_pool(name="outTp", bufs=2))
        outp = ctx.enter_context(tc.tile_pool(name="outp", bufs=2))

        for m in range(NMACRO):
            qT = qTp.tile([128, 8, QMACRO], F16, tag="qT")
            # ---- subtile stage: LN(x), q proj, l2norm, build qT ----
            for s in range(NSUB):
                r0 = m * QMACRO + s * 128
                x_t = xin.tile([128, DIM], F32, tag="x")
                nc.sync.dma_start(out=x_t[:], in_=XS[r0:r0 + 128, :])
                xn = xnp.tile([128, DIM], F16, tag="xn")
                _emit_layernorm(nc, small, xn[:], x_t[:], lig, lib_, eps_tile)
                xnT = xnTp.tile([128, 8, 128], F16, tag="xnT")
                for c0 in (0, 4):
                    ptr = ps_tr.tile([128, 4, 128], F16, tag="tr")
                    for t in range(4):
                        nc.tensor.transpose(ptr[:, t, :],
                                            xn[:, (c0 + t) * 128:(c0 + t + 1) * 128],
                                            ident[:])
                    nc.vector.tensor_copy(xnT[:, c0:c0 + 4, :], ptr[:])
                pq = ps_big.tile([128, DIM], F32, tag="big")
                for kc in range(8):
                    for half in range(2):
                        nc.tensor.matmul(
                            pq[:, half * 512:(half + 1) * 512],
                            lhsT=xnT[:, kc, :],
                            rhs=wq_sb[:, kc, half * 512:(half + 1) * 512],
                            start=(kc == 0), stop=(kc == 7))
                qf = qfp.tile([128, DIM], F16, tag="qf")
                _emit_l2norm_heads(nc, small, sq_pool,
                                   qf[:].rearrange("p (h d) -> p h d", d=HD),
                                   pq[:], qsc)
                for c0 in (0, 4):
                    ptr = ps_tr.tile([128, 4, 128], F16, tag="tr")
                    for t in range(4):
                        nc.tensor.transpose(ptr[:, t, :],
                                            qf[:, (c0 + t) * 128:(c0 + t + 1) * 128],
                                            ident[:])
                    nc.vector.tensor_copy(qT[:, c0:c0 + 4, s * 128:(s + 1) * 128],
                                          ptr[:])

            # ---- head stage ----
            outT = outTp.tile([128, 8, QMACRO], F16, tag="outT")
            for h in range(HEADS):
                c, j = h // 2, h % 2
                jb = j * HD
                kT_h = kT[jb:jb + HD, c, :]
                qT_h = qT[jb:jb + HD, c, :]
                ps_s = ps_sc.tile([128, 2, QMACRO], F32, tag="sc")
                for kc in range(2):
                    nc.tensor.matmul(ps_s[:, kc, :],
                                     lhsT=kT_h[:, kc * 128:(kc + 1) * 128],
                                     rhs=qT_h, start=True, stop=True)
                ps_n = ps_sm.tile([1, QMACRO], F32, tag="sm")
                nc.tensor.matmul(ps_n[:], lhsT=kT_h[:, T:T + 1],
                                 rhs=qT_h, start=True, stop=True)
                et = etp.tile([128, 2, QMACRO], F16, tag="et")
                for kc in range(2):
                    nc.scalar.activation(out=et[:, kc, :], in_=ps_s[:, kc, :],
                                         func=AF.Exp, bias=expb[:], scale=SCALE)
                en = etn.tile([1, QMACRO], F16, tag="en")
                nc.scalar.activation(out=en[0:1, :], in_=ps_n[0:1, :],
                                     func=AF.Exp, bias=expb[0:1, :], scale=SCALE)
                po = ps_o.tile([HD + 1, QMACRO], F32, tag="o")
                nc.tensor.matmul(po[:], lhsT=v_sb[:, 0, h, :],
                                 rhs=et[:, 0, :], start=True, stop=False)
                nc.tensor.matmul(po[:], lhsT=v_sb[:, 1, h, :],
                                 rhs=et[:, 1, :], start=False, stop=False)
                nc.tensor.matmul(po[:], lhsT=vnull[0:1, :],
                                 rhs=en[0:1, :], start=False, stop=True)
                sums = sumsp.tile([HD + 1, QMACRO], F16, tag="sums")
                nc.vector.tensor_copy(sums[HD:HD + 1, :], po[HD:HD + 1, :])
                pb = ps_sm.tile([HD, QMACRO], F32, tag="sm")
                nc.tensor.matmul(pb[:], lhsT=ones_t[HD:HD + 1, 0:HD],
                                 rhs=sums[HD:HD + 1, :], start=True, stop=True)
                rb = rbp.tile([HD, QMACRO], F32, tag="rb")
                nc.vector.reciprocal(rb[:], pb[:])
                if j == 0:
                    nc.vector.tensor_mul(outT[0:HD, c, :], po[0:HD, :], rb[:])
                else:
                    stg = stgp.tile([HD, QMACRO], F16, tag="stg")
                    nc.vector.tensor_mul(stg[:], po[0:HD, :], rb[:])
                    nc.sync.dma_start(out=outT[HD:128, c, :], in_=stg[:])

            # ---- output stage: Wo + LN out ----
            for s in range(NSUB):
                r0 = m * QMACRO + s * 128
                pf = ps_big.tile([128, DIM], F32, tag="big")
                for kc in range(8):
                    for half in range(2):
                        nc.tensor.matmul(
                            pf[:, half * 512:(half + 1) * 512],
                            lhsT=outT[:, kc, s * 128:(s + 1) * 128],
                            rhs=wo_sb[:, kc, half * 512:(half + 1) * 512],
                            start=(kc == 0), stop=(kc == 7))
                ob = outp.tile([128, DIM], F32, tag="ob")
                _emit_layernorm(nc, small, ob[:], pf[:], log_, lob, eps_tile)
                nc.sync.dma_start(out=OUT[r0:r0 + 128, :], in_=ob[:])

    nc.compile()
    return nc


_NC_CACHE = None


def kernel(**inputs):
    global _NC_CACHE
    if _NC_CACHE is None:
        _NC_CACHE = build_nc()
    nc = _NC_CACHE

    x = np.asarray(inputs["x"], np.float32)
    context = np.asarray(inputs["context"], np.float32)
    shared = {
        "Wq": np.asarray(inputs["Wq"], np.float32).astype(np.float16),
        "Wkv": np.asarray(inputs["Wkv"], np.float32).astype(np.float16),
        "Wo": np.asarray(inputs["Wo"], np.float32).astype(np.float16),
        "null_kv": np.asarray(inputs["null_kv"], np.float32),
        "q_scale": np.asarray(inputs["q_scale"], np.float32),
        "k_scale": np.asarray(inputs["k_scale"], np.float32),
        "ln_in_g": np.asarray(inputs["ln_in_g"], np.float32),
        "ln_in_b": np.asarray(inputs["ln_in_b"], np.float32),
        "ln_ctx_g": np.asarray(inputs["ln_ctx_g"], np.float32),
        "ln_ctx_b": np.asarray(inputs["ln_ctx_b"], np.float32),
        "ln_out_g": np.asarray(inputs["ln_out_g"], np.float32),
        "ln_out_b": np.asarray(inputs["ln_out_b"], np.float32),
    }
    B, N, _ = x.shape
    in_maps = []
    for c in range(N_CORES):
        b, n0 = c // 2, (c % 2) * ROWS
        in_maps.append({"xs": np.ascontiguousarray(x[b, n0:n0 + ROWS]),
                        "ctx": np.ascontiguousarray(context[b]), **shared})

    res = run_bass_kernel_spmd(nc, in_maps, list(range(N_CORES)))

    out = np.empty((B, N, DIM), np.float32)
    for c in range(N_CORES):
        b, n0 = c // 2, (c % 2) * ROWS
        out[b, n0:n0 + ROWS] = res.results[c]["out"]
    return out
